# revision 23
# baseline (speedup 1.0000x reference)
"""Multi-head causal attention (B=4,S=2048,D=1024,H=16,Dh=64) on 8 trn2 cores.

Sharding: core c -> batch b=c//2, head-group g=c%2 (8 heads each).
Data-parallel over B, tensor-parallel over heads: W_Q/K/V column-split,
W_out row-split; host sums the two partial outputs per batch and adds bias.

Per-core kernel, all matmuls in float32r (fp32 layout end-to-end, PE at full
rate). Scores are computed transposed ([k,q] tiles) so softmax needs no
on-chip transposes; V carries an extra ones-column per head so the A.V
accumulation also produces softmax denominators (max-subtraction is skipped:
|scores/8| < ~3 for these inputs). Causal masking: fully-masked k-blocks are
skipped; diagonal blocks get a [128,128] staircase mask add plus a zeroed
prefix, and exp runs only over the valid column range.

Schedule: the two heads of a pair share one two-bank PSUM tile per k-step
(score matmuls run concurrently in disjoint PE row groups, K=64 each; one
merged exp covers both), and QKV-projection/out-projection matmuls are
interleaved into the attention k-loop as PE fill work so exp latency never
idles the in-order PE stream.
"""
import numpy as np

N_CORES = 8
B, S, D = 4, 2048, 1024
HG = 512           # per-core slice of d_out (8 heads x 64)
NEG = -1.0e5       # causal mask add (exp(NEG/8) == 0 in fp32)

_CACHE = {}


def _build(iters=1):
    import concourse.bacc as bacc
    import concourse.mybir as mybir
    import concourse.tile as tile

    F32R = mybir.dt.float32r
    F32 = mybir.dt.float32
    EXP = mybir.ActivationFunctionType.Exp

    nc = bacc.Bacc(dynamic_dma_scratch_size=2048)
    xt = nc.declare_dram_parameter("xt", [D, S], F32R, isOutput=False)
    wq = nc.declare_dram_parameter("wq", [D, HG], F32R, isOutput=False)
    wk = nc.declare_dram_parameter("wk", [D, HG], F32R, isOutput=False)
    wv = nc.declare_dram_parameter("wv", [D, HG], F32R, isOutput=False)
    wo = nc.declare_dram_parameter("wo", [HG, D], F32R, isOutput=False)
    masks = nc.declare_dram_parameter("masks", [128, 128], F32, isOutput=False)
    zrv = nc.declare_dram_parameter("zrv", [128, 384], F32R, isOutput=False)
    onesv = nc.declare_dram_parameter("onesv", [128, 16, 8, 1], F32R, isOutput=False)
    out_t = nc.declare_dram_parameter("out_t", [D, S], F32, isOutput=True)

    def emit(tc):
        with tc.tile_pool(name="pp", bufs=1) as pp, \
             tc.tile_pool(name="pmm", bufs=1, space="PSUM") as pmm:
            KT = pp.tile([128, 4, S], F32R, tag="KT", name="KT")
            VE = pp.tile([128, 16, 8, 65], F32R, tag="VE", name="VE")
            MK = pp.tile([128, 128], F32, tag="MK", name="MK")
            ZR = pp.tile([128, 384], F32R, tag="ZR", name="ZR")
            WOS = pp.tile([128, 4, D], F32R, tag="WOS", name="WOS")
            wqs = pp.tile([128, 8, HG], F32R, tag="wqs", name="wqs")
            wks = pp.tile([128, 8, HG], F32R, tag="wks", name="wks")
            wvs = pp.tile([128, 8, HG], F32R, tag="wvs", name="wvs")
            nc.sync.dma_start(wqs[:], wq[:].rearrange("(t p) o -> p t o", p=128))
            nc.sync.dma_start(wks[:], wk[:].rearrange("(t p) o -> p t o", p=128))
            nc.sync.dma_start(wvs[:], wv[:].rearrange("(t p) o -> p t o", p=128))
            nc.sync.dma_start(MK[:], masks[:])
            nc.sync.dma_start(ZR[:], zrv[:])
            nc.sync.dma_start(VE[:, :, :, 64:65], onesv[:])
            nc.sync.dma_start(WOS[:], wo[:].rearrange("(t p) o -> p t o", p=128))
            xt_r = xt[:].rearrange("(t p) s -> p t s", p=128)

            # ---- fill-work generators (each yielded closure emits one PE
            # matmul group so attention can interleave them as PE fill) ----
            st_all = {}

            def make_load(sb):
                st8 = st_all.setdefault(sb, {})

                def load_xs():
                    st8["xs"] = pp.tile([128, 8, 512], F32R, tag="xs", bufs=2,
                                        name=f"xs{sb}")
                    nc.sync.dma_start(st8["xs"][:],
                                      xt_r[:, :, 512 * sb:512 * sb + 512])
                return load_xs

            def qkv_unit_lists(sb):
                """Returns (q_tiles, k_tiles, v_tiles) closure lists."""
                return (list(gen_q(sb)), list(gen_k(sb)), list(gen_v(sb)))

            def gen_qkv(sb):
                qs, ks, vs = qkv_unit_lists(sb)
                yield from qs
                yield from ks
                yield from vs

            def gen_q(sb):
                st8 = st_all[sb]

                for ot in range(4):
                    def q_tile(ot=ot):
                        xs = st8["xs"]
                        if "qtb" not in st8:
                            st8["qtb"] = pp.tile([128, 4, 512], F32R,
                                                 tag=f"qtb{sb % 2}",
                                                 name=f"qtb{sb}")
                        pq = pmm.tile([128, 512], F32, tag="mm512", bufs=2,
                                      name=f"pq{sb}_{ot}")
                        for it in range(8):
                            nc.tensor.matmul(
                                pq[:], wqs[:, it, 128 * ot:128 * ot + 128],
                                xs[:, it, :], start=(it == 0), stop=(it == 7))
                        nc.vector.tensor_copy(st8["qtb"][:, ot, :], pq[:])
                    yield q_tile

            def gen_k(sb):
                st8 = st_all[sb]
                for ot in range(4):
                    def k_tile(ot=ot):
                        xs = st8["xs"]
                        pk = pmm.tile([128, 512], F32, tag="mm512", bufs=2,
                                      name=f"pk{sb}_{ot}")
                        for it in range(8):
                            nc.tensor.matmul(
                                pk[:], wks[:, it, 128 * ot:128 * ot + 128],
                                xs[:, it, :], start=(it == 0), stop=(it == 7))
                        nc.vector.tensor_copy(
                            KT[:, ot, 512 * sb:512 * sb + 512], pk[:])
                    yield k_tile

            def gen_v(sb):
                st8 = st_all[sb]
                for st in range(4):
                    def v_tile(st=st):
                        xs = st8["xs"]
                        pv = pmm.tile([128, 512], F32, tag="mm512", bufs=2,
                                      name=f"pv{sb}_{st}")
                        for it in range(8):
                            nc.tensor.matmul(
                                pv[:], xs[:, it, 128 * st:128 * st + 128],
                                wvs[:, it, :], start=(it == 0), stop=(it == 7))
                        nc.vector.tensor_copy(
                            VE[:, 4 * sb + st, :, 0:64],
                            pv[:].rearrange("p (h d) -> p h d", d=64))
                    yield v_tile

            def gen_outproj(qb, call):
                qsl = slice(512 * qb, 512 * qb + 512)
                for ot in range(8):
                    def f(ot=ot):
                        po = pmm.tile([128, 512], F32, tag="mm512", bufs=2,
                                      name=f"po{qb}_{ot}")
                        for dt in range(4):
                            nc.tensor.matmul(
                                po[:], WOS[:, dt, 128 * ot:128 * ot + 128],
                                call[:, dt, :], start=(dt == 0), stop=(dt == 3))
                        so = pp.tile([128, 512], F32, tag="so", bufs=2,
                                     name=f"so{qb}_{ot}")
                        nc.vector.tensor_copy(so[:], po[:])
                        nc.sync.dma_start(out_t[128 * ot:128 * ot + 128, qsl],
                                          so[:])
                    yield f

            def run_qkv(sb):
                for u in gen_qkv(sb):
                    u()

            def emit_attention(qb, qtb, fills):
                nkt = 4 * qb + 4
                calls = pp.tile([128, 4, 512], F32R, tag=f"call{qb % 2}",
                                name=f"call{qb}")
                n_steps = 4 * (nkt + 1)
                fi = [0]

                def pop_fills(step_idx):
                    want = int(round(len(fills) * (step_idx + 1) / n_steps))
                    while fi[0] < min(want, len(fills)):
                        fills[fi[0]]()
                        fi[0] += 1

                step = 0
                for pr in range(4):           # head pair (2pr, 2pr+1)
                    cext = None
                    pts = {}
                    for kt in range(nkt + 1):
                        if kt < nkt:
                            if kt == 0:
                                cext = pmm.tile([65, 1024], F32, tag="cext",
                                                bufs=1, name=f"ce{qb}_{pr}")
                            sc = pmm.tile([128, 1024], F32, tag="sc1024",
                                          bufs=2, name=f"sc{qb}{pr}{kt}")
                            ksl = slice(128 * kt, 128 * kt + 128)
                            nc.tensor.matmul(sc[:, 0:512], KT[0:64, pr, ksl],
                                             qtb[0:64, pr, :],
                                             start=True, stop=True)
                            nc.tensor.matmul(sc[:, 512:1024],
                                             KT[64:128, pr, ksl],
                                             qtb[64:128, pr, :],
                                             start=True, stop=True)
                            sc3 = sc[:].rearrange("p (s c) -> p s c", s=2)
                            j = kt - 4 * qb
                            pt = pp.tile([128, 1024], F32R, tag="pt", bufs=3,
                                         name=f"p{qb}{pr}{kt}")
                            p3 = pt[:].rearrange("p (s c) -> p s c", s=2)
                            if j >= 0:   # diagonal: staircase mask + prefix 0
                                nc.vector.tensor_add(
                                    sc3[:, :, 128 * j:128 * j + 128],
                                    sc3[:, :, 128 * j:128 * j + 128],
                                    MK[:, None, :].broadcast_to((128, 2, 128)))
                                if j > 0:
                                    nc.vector.tensor_copy(
                                        p3[:, :, 0:128 * j],
                                        ZR[:, None, 0:128 * j].broadcast_to(
                                            (128, 2, 128 * j)))
                                nc.scalar.activation(
                                    p3[:, :, 128 * j:512],
                                    sc3[:, :, 128 * j:512], EXP, scale=0.125)
                            else:
                                nc.scalar.activation(pt[:], sc[:], EXP,
                                                     scale=0.125)
                            pts[kt] = pt
                        pop_fills(step)
                        step += 1
                        if kt >= 1:
                            pt = pts.pop(kt - 1)
                            nc.tensor.matmul(
                                cext[:, 0:512], VE[:, kt - 1, 2 * pr, :],
                                pt[:, 0:512],
                                start=(kt - 1 == 0), stop=(kt - 1 == nkt - 1),
                                skip_group_check=True)
                            nc.tensor.matmul(
                                cext[:, 512:1024], VE[:, kt - 1, 2 * pr + 1, :],
                                pt[:, 512:1024],
                                start=(kt - 1 == 0), stop=(kt - 1 == nkt - 1),
                                skip_group_check=True)
                    rcA = pp.tile([1, 512], F32, tag="recip", bufs=2,
                                  name=f"ra{qb}{pr}")
                    rcB = pp.tile([1, 512], F32, tag="recip", bufs=2,
                                  name=f"rb{qb}{pr}")
                    nc.vector.reciprocal(rcA[:], cext[64:65, 0:512])
                    bcA = pp.tile([64, 512], F32, tag="bc", bufs=2,
                                  name=f"ba{qb}{pr}")
                    nc.gpsimd.partition_broadcast(bcA[:], rcA[:])
                    nc.vector.reciprocal(rcB[:], cext[64:65, 512:1024])
                    bcB = pp.tile([64, 512], F32, tag="bc", bufs=2,
                                  name=f"bb{qb}{pr}")
                    nc.gpsimd.partition_broadcast(bcB[:], rcB[:])
                    nc.vector.tensor_mul(calls[0:64, pr, :],
                                         cext[0:64, 0:512], bcA[:])
                    nc.vector.tensor_mul(calls[64:128, pr, :],
                                         cext[0:64, 512:1024], bcB[:])
                while fi[0] < len(fills):
                    fills[fi[0]]()
                    fi[0] += 1
                return calls

            # ---------------- main schedule ----------------
            make_load(0)()
            qs0, ks0, vs0 = qkv_unit_lists(0)
            # essentials for attention(0) pair 0: Q-tile 0, K-tile 0, all V
            for u in (qs0[0], ks0[0], *vs0):
                u()
            make_load(1)()          # prefetch: overlaps attention(0)
            pre_fills = [qs0[1], ks0[1], qs0[2], ks0[2], qs0[3], ks0[3]]
            calls = {}
            for sb in range(4):
                fills = list(pre_fills)
                pre_fills = []
                if sb < 3:
                    fills += list(gen_qkv(sb + 1))
                if sb < 2:
                    fills.append(make_load(sb + 2))
                if sb >= 1:
                    fills += list(gen_outproj(sb - 1, calls[sb - 1]))
                calls[sb] = emit_attention(sb, st_all[sb]["qtb"], fills)
            for u in gen_outproj(3, calls[3]):
                u()

    with tile.TileContext(nc) as tc:
        if iters == 1:
            emit(tc)
        else:
            engs = (mybir.EngineType.PE, mybir.EngineType.Activation,
                    mybir.EngineType.DVE, mybir.EngineType.SP,
                    mybir.EngineType.Pool)
            with tc.For_i(0, iters, 1, hint_engines=engs):
                emit(tc)
    nc.compile()
    return nc


class _Runner:
    """Persistent jitted SPMD executor (mirrors bass2jax.run_bass_via_pjrt,
    but reusable across calls without retracing)."""

    def __init__(self, nc, n_cores):
        import jax
        import concourse.mybir as mybir
        from jax.experimental.shard_map import shard_map
        from jax.sharding import Mesh, PartitionSpec
        from concourse.bass2jax import (
            _bass_exec_p, install_neuronx_cc_hook, partition_id_tensor)

        install_neuronx_cc_hook()
        self.jax = jax
        self.n_cores = n_cores
        pname = nc.partition_id_tensor.name if nc.partition_id_tensor else None
        in_names, out_names, out_avals, zero_outs = [], [], [], []
        for alloc in nc.m.functions[0].allocations:
            if not isinstance(alloc, mybir.MemoryLocationSet):
                continue
            name = alloc.memorylocations[0].name
            if alloc.kind == "ExternalInput":
                if name != pname:
                    in_names.append(name)
            elif alloc.kind == "ExternalOutput":
                shape = tuple(alloc.tensor_shape)
                dtype = mybir.dt.np(alloc.dtype)
                out_names.append(name)
                out_avals.append(jax.core.ShapedArray(shape, dtype))
                zero_outs.append(np.zeros(shape, dtype))
        self.in_names, self.out_names = in_names, out_names
        self.out_avals, self.zero_outs = out_avals, zero_outs
        n_params, n_outs = len(in_names), len(out_avals)
        all_in = in_names + out_names + ([pname] if pname else [])

        def _body(*args):
            operands = list(args)
            if pname is not None:
                operands.append(partition_id_tensor())
            return tuple(_bass_exec_p.bind(
                *operands, out_avals=tuple(out_avals), in_names=tuple(all_in),
                out_names=tuple(out_names), lowering_input_output_aliases=(),
                sim_require_finite=True, sim_require_nnan=True, nc=nc))

        devices = [d for d in jax.devices() if d.platform != "cpu"]
        if len(devices) < n_cores:
            try:
                devices = list(jax.devices("axon"))
            except Exception:
                devices = []
        if len(devices) < n_cores:
            try:
                jax.config.update("jax_platforms", "axon,cpu")
                devices = list(jax.devices("axon"))
            except Exception:
                devices = list(jax.devices())
        devices = devices[:n_cores]
        self.mesh = Mesh(np.asarray(devices), ("core",))
        in_specs = (PartitionSpec("core"),) * (n_params + n_outs)
        out_specs = (PartitionSpec("core"),) * n_outs
        self.fn = jax.jit(
            shard_map(_body, mesh=self.mesh, in_specs=in_specs,
                      out_specs=out_specs, check_rep=False),
            keep_unused=True)
        self._zeros_dev = None

    def prep(self, in_maps):
        from jax.sharding import NamedSharding, PartitionSpec
        sh = NamedSharding(self.mesh, PartitionSpec("core"))
        args = [
            self.jax.device_put(
                np.concatenate([np.asarray(in_maps[c][nm])
                                for c in range(self.n_cores)], axis=0), sh)
            for nm in self.in_names
        ]
        if self._zeros_dev is None:
            self._zeros_dev = [
                self.jax.device_put(
                    np.zeros((self.n_cores * z.shape[0], *z.shape[1:]), z.dtype),
                    sh)
                for z in self.zero_outs
            ]
        return args + self._zeros_dev

    def run_dev(self, dev_args):
        return self.fn(*dev_args)

    def run(self, in_maps):
        outs = self.run_dev(self.prep(in_maps))
        res = []
        for c in range(self.n_cores):
            res.append({
                nm: np.asarray(outs[i]).reshape(
                    self.n_cores, *self.out_avals[i].shape)[c]
                for i, nm in enumerate(self.out_names)})
        return res


def _make_masks():
    p = np.arange(128)[:, None]
    c = np.arange(128)[None, :]
    return np.where(c >= p, 0.0, NEG).astype(np.float32)


def _in_maps(X, W_Q, W_K, W_V, W_out):
    masks = _make_masks()
    ones = np.ones((128, 16, 8, 1), np.float32)
    zeros = np.zeros((128, 384), np.float32)
    maps = []
    for c in range(N_CORES):
        b, g = c // 2, c % 2
        sl = slice(HG * g, HG * g + HG)
        maps.append({
            "xt": np.ascontiguousarray(X[b].T),
            "wq": np.ascontiguousarray(W_Q[:, sl]),
            "wk": np.ascontiguousarray(W_K[:, sl]),
            "wv": np.ascontiguousarray(W_V[:, sl]),
            "wo": np.ascontiguousarray(W_out[sl, :]),
            "masks": masks,
            "zrv": zeros,
            "onesv": ones,
        })
    return maps


def get_runner(iters=1):
    key = ("runner", iters)
    if key not in _CACHE:
        _CACHE[key] = _Runner(_build(iters), N_CORES)
    return _CACHE[key]


def kernel(X, W_K, W_Q, W_V, W_out, b_out):
    X = np.asarray(X, np.float32)
    r = get_runner()
    res = r.run(_in_maps(X, np.asarray(W_Q, np.float32),
                         np.asarray(W_K, np.float32),
                         np.asarray(W_V, np.float32), W_out))
    out = np.empty((B, S, D), np.float32)
    bo = np.asarray(b_out, np.float32)
    for b in range(B):
        out[b] = res[2 * b]["out_t"].T + res[2 * b + 1]["out_t"].T + bo
    return out


# revision 24
# speedup vs baseline: 1.3786x; 1.3786x over previous
"""Multi-head causal attention (B=4,S=2048,D=1024,H=16,Dh=64) on 8 trn2 cores.

Sharding: core c -> batch b=c//2, head-group g=c%2 (8 heads each).
Data-parallel over B, tensor-parallel over heads: W_Q/K/V column-split,
W_out row-split; host sums the two partial outputs per batch and adds bias.

Per-core kernel, all matmuls in float32r (fp32 layout end-to-end, PE at full
rate). Scores are computed transposed ([k,q] tiles) so softmax needs no
on-chip transposes; V carries an extra ones-column per head so the A.V
accumulation also produces softmax denominators (max-subtraction is skipped:
|scores/8| < ~3 for these inputs). Causal masking: fully-masked k-blocks are
skipped; diagonal blocks get a [128,128] staircase mask add plus a zeroed
prefix, and exp runs only over the valid column range.

Schedule: the two heads of a pair share one two-bank PSUM tile per k-step
(score matmuls run concurrently in disjoint PE row groups, K=64 each; one
merged exp covers both), and QKV-projection/out-projection matmuls are
interleaved into the attention k-loop as PE fill work so exp latency never
idles the in-order PE stream.
"""
import numpy as np

N_CORES = 8
B, S, D = 4, 2048, 1024
HG = 512           # per-core slice of d_out (8 heads x 64)
NEG = -1.0e5       # causal mask add (exp(NEG/8) == 0 in fp32)

_CACHE = {}


def _build(iters=1):
    import concourse.bacc as bacc
    import concourse.mybir as mybir
    import concourse.tile as tile

    F32R = mybir.dt.float32r
    F32 = mybir.dt.float32
    EXP = mybir.ActivationFunctionType.Exp

    nc = bacc.Bacc(dynamic_dma_scratch_size=2048)
    xt = nc.declare_dram_parameter("xt", [D, S], F32R, isOutput=False)
    wq = nc.declare_dram_parameter("wq", [D, HG], F32R, isOutput=False)
    wk = nc.declare_dram_parameter("wk", [D, HG], F32R, isOutput=False)
    wv = nc.declare_dram_parameter("wv", [D, HG], F32R, isOutput=False)
    wo = nc.declare_dram_parameter("wo", [HG, D], F32R, isOutput=False)
    masks = nc.declare_dram_parameter("masks", [128, 128], F32, isOutput=False)
    zrv = nc.declare_dram_parameter("zrv", [128, 384], F32R, isOutput=False)
    onesv = nc.declare_dram_parameter("onesv", [128, 16, 8, 1], F32R, isOutput=False)
    out_t = nc.declare_dram_parameter("out_t", [D, S], F32, isOutput=True)

    def emit(tc):
        with tc.tile_pool(name="pp", bufs=1) as pp, \
             tc.tile_pool(name="pmm", bufs=1, space="PSUM") as pmm:
            KT = pp.tile([128, 4, S], F32R, tag="KT", name="KT")
            VE = pp.tile([128, 16, 8, 65], F32R, tag="VE", name="VE")
            MK = pp.tile([128, 128], F32, tag="MK", name="MK")
            ZR = pp.tile([128, 384], F32R, tag="ZR", name="ZR")
            WOS = pp.tile([128, 4, D], F32R, tag="WOS", name="WOS")
            wqs = pp.tile([128, 8, HG], F32R, tag="wqs", name="wqs")
            wks = pp.tile([128, 8, HG], F32R, tag="wks", name="wks")
            wvs = pp.tile([128, 8, HG], F32R, tag="wvs", name="wvs")
            nc.sync.dma_start(wqs[:], wq[:].rearrange("(t p) o -> p t o", p=128))
            nc.sync.dma_start(wks[:], wk[:].rearrange("(t p) o -> p t o", p=128))
            nc.sync.dma_start(wvs[:], wv[:].rearrange("(t p) o -> p t o", p=128))
            nc.sync.dma_start(MK[:], masks[:])
            nc.sync.dma_start(ZR[:], zrv[:])
            nc.sync.dma_start(VE[:, :, :, 64:65], onesv[:])
            nc.sync.dma_start(WOS[:], wo[:].rearrange("(t p) o -> p t o", p=128))
            xt_r = xt[:].rearrange("(t p) s -> p t s", p=128)

            # ---- fill-work generators (each yielded closure emits one PE
            # matmul group so attention can interleave them as PE fill) ----
            st_all = {}

            def make_load(sb):
                st8 = st_all.setdefault(sb, {})

                def load_xs():
                    st8["xs"] = pp.tile([128, 8, 512], F32R, tag="xs", bufs=2,
                                        name=f"xs{sb}")
                    nc.sync.dma_start(st8["xs"][:],
                                      xt_r[:, :, 512 * sb:512 * sb + 512])
                return load_xs

            def qkv_unit_lists(sb):
                """Returns (q_tiles, k_tiles, v_tiles) closure lists."""
                return (list(gen_q(sb)), list(gen_k(sb)), list(gen_v(sb)))

            def gen_qkv(sb):
                qs, ks, vs = qkv_unit_lists(sb)
                yield from qs
                yield from ks
                yield from vs

            def gen_q(sb):
                st8 = st_all[sb]

                for ot in range(4):
                    def q_tile(ot=ot):
                        xs = st8["xs"]
                        if "qtb" not in st8:
                            st8["qtb"] = pp.tile([128, 4, 512], F32R,
                                                 tag=f"qtb{sb % 2}",
                                                 name=f"qtb{sb}")
                        pq = pmm.tile([128, 512], F32, tag="mm512", bufs=2,
                                      name=f"pq{sb}_{ot}")
                        for it in range(8):
                            nc.tensor.matmul(
                                pq[:], wqs[:, it, 128 * ot:128 * ot + 128],
                                xs[:, it, :], start=(it == 0), stop=(it == 7))
                        nc.vector.tensor_copy(st8["qtb"][:, ot, :], pq[:])
                    yield q_tile

            def gen_k(sb):
                st8 = st_all[sb]
                for ot in range(4):
                    def k_tile(ot=ot):
                        xs = st8["xs"]
                        pk = pmm.tile([128, 512], F32, tag="mm512", bufs=2,
                                      name=f"pk{sb}_{ot}")
                        for it in range(8):
                            nc.tensor.matmul(
                                pk[:], wks[:, it, 128 * ot:128 * ot + 128],
                                xs[:, it, :], start=(it == 0), stop=(it == 7))
                        nc.vector.tensor_copy(
                            KT[:, ot, 512 * sb:512 * sb + 512], pk[:])
                    yield k_tile

            def gen_v(sb):
                st8 = st_all[sb]
                for st in range(4):
                    def v_tile(st=st):
                        xs = st8["xs"]
                        pv = pmm.tile([128, 512], F32, tag="mm512", bufs=2,
                                      name=f"pv{sb}_{st}")
                        for it in range(8):
                            nc.tensor.matmul(
                                pv[:], xs[:, it, 128 * st:128 * st + 128],
                                wvs[:, it, :], start=(it == 0), stop=(it == 7))
                        nc.vector.tensor_copy(
                            VE[:, 4 * sb + st, :, 0:64],
                            pv[:].rearrange("p (h d) -> p h d", d=64))
                    yield v_tile

            def gen_outproj(qb, call):
                qsl = slice(512 * qb, 512 * qb + 512)
                for ot in range(8):
                    def f(ot=ot):
                        po = pmm.tile([128, 512], F32, tag="mm512", bufs=2,
                                      name=f"po{qb}_{ot}")
                        for dt in range(4):
                            nc.tensor.matmul(
                                po[:], WOS[:, dt, 128 * ot:128 * ot + 128],
                                call[:, dt, :], start=(dt == 0), stop=(dt == 3))
                        so = pp.tile([128, 512], F32, tag="so", bufs=2,
                                     name=f"so{qb}_{ot}")
                        nc.vector.tensor_copy(so[:], po[:])
                        nc.sync.dma_start(out_t[128 * ot:128 * ot + 128, qsl],
                                          so[:])
                    yield f

            def run_qkv(sb):
                for u in gen_qkv(sb):
                    u()

            def emit_attention(qb, qtb, fills):
                nkt = 4 * qb + 4
                calls = pp.tile([128, 4, 512], F32R, tag=f"call{qb % 2}",
                                name=f"call{qb}")
                n_steps = 4 * (nkt + 1)
                fi = [0]

                def pop_fills(step_idx):
                    want = int(round(len(fills) * (step_idx + 1) / n_steps))
                    while fi[0] < min(want, len(fills)):
                        fills[fi[0]]()
                        fi[0] += 1

                step = 0
                for pr in range(4):           # head pair (2pr, 2pr+1)
                    cext = None
                    pts = {}
                    for kt in range(nkt + 1):
                        if kt < nkt:
                            if kt == 0:
                                cext = pmm.tile([65, 1024], F32, tag="cext",
                                                bufs=1, name=f"ce{qb}_{pr}")
                            sc = pmm.tile([128, 1024], F32, tag="sc1024",
                                          bufs=2, name=f"sc{qb}{pr}{kt}")
                            ksl = slice(128 * kt, 128 * kt + 128)
                            nc.tensor.matmul(sc[:, 0:512], KT[0:64, pr, ksl],
                                             qtb[0:64, pr, :],
                                             start=True, stop=True)
                            nc.tensor.matmul(sc[:, 512:1024],
                                             KT[64:128, pr, ksl],
                                             qtb[64:128, pr, :],
                                             start=True, stop=True)
                            sc3 = sc[:].rearrange("p (s c) -> p s c", s=2)
                            j = kt - 4 * qb
                            pt = pp.tile([128, 1024], F32R, tag="pt", bufs=3,
                                         name=f"p{qb}{pr}{kt}")
                            p3 = pt[:].rearrange("p (s c) -> p s c", s=2)
                            if j >= 0:   # diagonal: staircase mask + prefix 0
                                nc.vector.tensor_add(
                                    sc3[:, :, 128 * j:128 * j + 128],
                                    sc3[:, :, 128 * j:128 * j + 128],
                                    MK[:, None, :].broadcast_to((128, 2, 128)))
                                if j > 0:
                                    nc.vector.tensor_copy(
                                        p3[:, :, 0:128 * j],
                                        ZR[:, None, 0:128 * j].broadcast_to(
                                            (128, 2, 128 * j)))
                                nc.scalar.activation(
                                    p3[:, :, 128 * j:512],
                                    sc3[:, :, 128 * j:512], EXP, scale=0.125)
                            else:
                                nc.scalar.activation(pt[:], sc[:], EXP,
                                                     scale=0.125)
                            pts[kt] = pt
                        pop_fills(step)
                        step += 1
                        if kt >= 1:
                            pt = pts.pop(kt - 1)
                            nc.tensor.matmul(
                                cext[:, 0:512], VE[:, kt - 1, 2 * pr, :],
                                pt[:, 0:512],
                                start=(kt - 1 == 0), stop=(kt - 1 == nkt - 1),
                                skip_group_check=True)
                            nc.tensor.matmul(
                                cext[:, 512:1024], VE[:, kt - 1, 2 * pr + 1, :],
                                pt[:, 512:1024],
                                start=(kt - 1 == 0), stop=(kt - 1 == nkt - 1),
                                skip_group_check=True)
                    recip = pp.tile([1, 1024], F32, tag="recip", bufs=1,
                                    name=f"rc{qb}{pr}")
                    nc.vector.reciprocal(recip[:], cext[64:65, :])
                    bc = pp.tile([64, 1024], F32, tag="bc", bufs=1,
                                 name=f"bc{qb}{pr}")
                    nc.gpsimd.partition_broadcast(bc[:], recip[:])
                    nc.vector.tensor_mul(calls[0:64, pr, :],
                                         cext[0:64, 0:512], bc[:, 0:512])
                    nc.vector.tensor_mul(calls[64:128, pr, :],
                                         cext[0:64, 512:1024], bc[:, 512:1024])
                while fi[0] < len(fills):
                    fills[fi[0]]()
                    fi[0] += 1
                return calls

            # ---------------- main schedule ----------------
            make_load(0)()
            qs0, ks0, vs0 = qkv_unit_lists(0)
            # essentials for attention(0) pair 0: Q-tile 0, K-tile 0, all V
            for u in (qs0[0], ks0[0], *vs0):
                u()
            make_load(1)()          # prefetch: overlaps attention(0)
            pre_fills = [qs0[1], ks0[1], qs0[2], ks0[2], qs0[3], ks0[3]]
            calls = {}
            for sb in range(4):
                fills = list(pre_fills)
                pre_fills = []
                if sb < 3:
                    fills += list(gen_qkv(sb + 1))
                if sb < 2:
                    fills.append(make_load(sb + 2))
                if sb >= 1:
                    fills += list(gen_outproj(sb - 1, calls[sb - 1]))
                calls[sb] = emit_attention(sb, st_all[sb]["qtb"], fills)
            for u in gen_outproj(3, calls[3]):
                u()

    with tile.TileContext(nc) as tc:
        if iters == 1:
            emit(tc)
        else:
            engs = (mybir.EngineType.PE, mybir.EngineType.Activation,
                    mybir.EngineType.DVE, mybir.EngineType.SP,
                    mybir.EngineType.Pool)
            with tc.For_i(0, iters, 1, hint_engines=engs):
                emit(tc)
    nc.compile()
    return nc


class _Runner:
    """Persistent jitted SPMD executor (mirrors bass2jax.run_bass_via_pjrt,
    but reusable across calls without retracing)."""

    def __init__(self, nc, n_cores):
        import jax
        import concourse.mybir as mybir
        from jax.experimental.shard_map import shard_map
        from jax.sharding import Mesh, PartitionSpec
        from concourse.bass2jax import (
            _bass_exec_p, install_neuronx_cc_hook, partition_id_tensor)

        install_neuronx_cc_hook()
        self.jax = jax
        self.n_cores = n_cores
        pname = nc.partition_id_tensor.name if nc.partition_id_tensor else None
        in_names, out_names, out_avals, zero_outs = [], [], [], []
        for alloc in nc.m.functions[0].allocations:
            if not isinstance(alloc, mybir.MemoryLocationSet):
                continue
            name = alloc.memorylocations[0].name
            if alloc.kind == "ExternalInput":
                if name != pname:
                    in_names.append(name)
            elif alloc.kind == "ExternalOutput":
                shape = tuple(alloc.tensor_shape)
                dtype = mybir.dt.np(alloc.dtype)
                out_names.append(name)
                out_avals.append(jax.core.ShapedArray(shape, dtype))
                zero_outs.append(np.zeros(shape, dtype))
        self.in_names, self.out_names = in_names, out_names
        self.out_avals, self.zero_outs = out_avals, zero_outs
        n_params, n_outs = len(in_names), len(out_avals)
        all_in = in_names + out_names + ([pname] if pname else [])

        def _body(*args):
            operands = list(args)
            if pname is not None:
                operands.append(partition_id_tensor())
            return tuple(_bass_exec_p.bind(
                *operands, out_avals=tuple(out_avals), in_names=tuple(all_in),
                out_names=tuple(out_names), lowering_input_output_aliases=(),
                sim_require_finite=True, sim_require_nnan=True, nc=nc))

        devices = [d for d in jax.devices() if d.platform != "cpu"]
        if len(devices) < n_cores:
            try:
                devices = list(jax.devices("axon"))
            except Exception:
                devices = []
        if len(devices) < n_cores:
            try:
                jax.config.update("jax_platforms", "axon,cpu")
                devices = list(jax.devices("axon"))
            except Exception:
                devices = list(jax.devices())
        devices = devices[:n_cores]
        self.mesh = Mesh(np.asarray(devices), ("core",))
        in_specs = (PartitionSpec("core"),) * (n_params + n_outs)
        out_specs = (PartitionSpec("core"),) * n_outs
        self.fn = jax.jit(
            shard_map(_body, mesh=self.mesh, in_specs=in_specs,
                      out_specs=out_specs, check_rep=False),
            keep_unused=True)
        self._zeros_dev = None

    def prep(self, in_maps):
        from jax.sharding import NamedSharding, PartitionSpec
        sh = NamedSharding(self.mesh, PartitionSpec("core"))
        args = [
            self.jax.device_put(
                np.concatenate([np.asarray(in_maps[c][nm])
                                for c in range(self.n_cores)], axis=0), sh)
            for nm in self.in_names
        ]
        if self._zeros_dev is None:
            self._zeros_dev = [
                self.jax.device_put(
                    np.zeros((self.n_cores * z.shape[0], *z.shape[1:]), z.dtype),
                    sh)
                for z in self.zero_outs
            ]
        return args + self._zeros_dev

    def run_dev(self, dev_args):
        return self.fn(*dev_args)

    def run(self, in_maps):
        outs = self.run_dev(self.prep(in_maps))
        res = []
        for c in range(self.n_cores):
            res.append({
                nm: np.asarray(outs[i]).reshape(
                    self.n_cores, *self.out_avals[i].shape)[c]
                for i, nm in enumerate(self.out_names)})
        return res


def _make_masks():
    p = np.arange(128)[:, None]
    c = np.arange(128)[None, :]
    return np.where(c >= p, 0.0, NEG).astype(np.float32)


def _in_maps(X, W_Q, W_K, W_V, W_out):
    masks = _make_masks()
    ones = np.ones((128, 16, 8, 1), np.float32)
    zeros = np.zeros((128, 384), np.float32)
    maps = []
    for c in range(N_CORES):
        b, g = c // 2, c % 2
        sl = slice(HG * g, HG * g + HG)
        maps.append({
            "xt": np.ascontiguousarray(X[b].T),
            "wq": np.ascontiguousarray(W_Q[:, sl]),
            "wk": np.ascontiguousarray(W_K[:, sl]),
            "wv": np.ascontiguousarray(W_V[:, sl]),
            "wo": np.ascontiguousarray(W_out[sl, :]),
            "masks": masks,
            "zrv": zeros,
            "onesv": ones,
        })
    return maps


def get_runner(iters=1):
    key = ("runner", iters)
    if key not in _CACHE:
        _CACHE[key] = _Runner(_build(iters), N_CORES)
    return _CACHE[key]


def kernel(X, W_K, W_Q, W_V, W_out, b_out):
    X = np.asarray(X, np.float32)
    r = get_runner()
    res = r.run(_in_maps(X, np.asarray(W_Q, np.float32),
                         np.asarray(W_K, np.float32),
                         np.asarray(W_V, np.float32), W_out))
    out = np.empty((B, S, D), np.float32)
    bo = np.asarray(b_out, np.float32)
    for b in range(B):
        out[b] = res[2 * b]["out_t"].T + res[2 * b + 1]["out_t"].T + bo
    return out
